# revision 1
# baseline (speedup 1.0000x reference)
"""MoE layer (softmax router, top-k, per-expert FFN D->F->gelu->D) on 8 TRN2 cores.

Strategy: expert-parallel. The router (a tiny T x E matmul + top-k) runs on the
host and doubles as the sharding function: core e receives exactly the tokens
whose top-k set contains expert e (the "all-to-all dispatch"), already
transposed to (D, C) so both device matmuls need no on-chip transposes.
Each core holds one expert's weights (pre-transposed on host, cast to bf16),
computes gelu(x @ W1^T + b1) @ W2^T, scales each token's output by its
renormalized gate on-chip, and the host scatter-adds the per-expert results
back into the full (T, D) output. b2 and the aux load-balance loss are added
on the host (b2 enters linearly: sum_e gate * b2[e]).
"""

import numpy as np
import ml_dtypes

import concourse.bass as bass  # noqa: F401  (registers AP machinery)
import concourse.mybir as mybir
import concourse.tile as tile
from concourse import bacc
from concourse.bass_utils import run_bass_kernel_spmd

P = 128
CB = 512  # token-block: free dim of layer-1 matmuls, partition chunk of layer-2
LB_WEIGHT = 0.01
N_CORES = 8

BF16 = mybir.dt.bfloat16
F32 = mybir.dt.float32
BF16_NP = ml_dtypes.bfloat16

_PROGRAM_CACHE: dict = {}


def _token_blocks(C: int) -> list[int]:
    blocks = [CB] * (C // CB)
    if C % CB:
        blocks.append(C % CB)
    return blocks


def _build_program(D: int, F: int, C: int):
    """One-expert FFN over C (padded) tokens; SPMD-identical across cores."""
    assert D % P == 0 and F % P == 0 and C % P == 0 and D % 512 == 0
    KD, MF, ND = D // P, F // P, D // 512
    blocks = _token_blocks(C)
    GELU = mybir.ActivationFunctionType.Gelu_apprx_tanh

    nc = bacc.Bacc(None, target_bir_lowering=False, debug=False)
    xT = nc.declare_dram_parameter("xT", [D, C], BF16, isOutput=False)
    w1t = nc.declare_dram_parameter("w1t", [D, F], BF16, isOutput=False)
    w2t = nc.declare_dram_parameter("w2t", [F, D], BF16, isOutput=False)
    b1 = nc.declare_dram_parameter("b1", [MF, P], F32, isOutput=False)
    gate = nc.declare_dram_parameter("gate", [C // P, P], F32, isOutput=False)
    y = nc.declare_dram_parameter("y", [C, D], F32, isOutput=True)

    w1v = w1t.rearrange("(k p) f -> k p f", p=P)
    w2v = w2t.rearrange("(k p) d -> k p d", p=P)
    xv = xT.rearrange("(k p) c -> k p c", p=P)
    yv = y.rearrange("(t p) d -> t p d", p=P)

    with tile.TileContext(nc) as tc:
        with (
            tc.tile_pool(name="const", bufs=1) as constp,
            tc.tile_pool(name="wres", bufs=1) as wres,
            tc.tile_pool(name="xres", bufs=1) as xres,
            tc.tile_pool(name="htp", bufs=1) as htp,
            tc.tile_pool(name="ysb", bufs=3) as yp,
            tc.tile_pool(name="psA", bufs=3, space="PSUM") as psA,
            tc.tile_pool(name="psB", bufs=2, space="PSUM") as psB,
        ):
            b1_sb = constp.tile([P, MF], F32, tag="b1")
            nc.sync.dma_start(out=b1_sb[:], in_=b1.rearrange("m p -> p m"))
            gate_sb = constp.tile([P, C // P], F32, tag="gate")
            nc.sync.dma_start(out=gate_sb[:], in_=gate.rearrange("t p -> p t"))

            x_sb = []
            for k in range(KD):
                t = xres.tile([P, C], BF16, tag=f"x{k}")
                nc.sync.dma_start(out=t[:], in_=xv[k])
                x_sb.append(t)
            w1_sb = []
            for k in range(KD):
                t = wres.tile([P, F], BF16, tag=f"w1_{k}")
                nc.sync.dma_start(out=t[:], in_=w1v[k])
                w1_sb.append(t)
            w2_sb = []
            for k in range(MF):
                t = wres.tile([P, D], BF16, tag=f"w2_{k}")
                nc.sync.dma_start(out=t[:], in_=w2v[k])
                w2_sb.append(t)

            c0 = 0
            for cb in blocks:
                # Layer 1: hT[f, c] = gelu(sum_d w1t[d, f] * xT[d, c] + b1[f])
                ht = htp.tile([P, MF, CB], BF16, tag="ht")
                for m in range(MF):
                    ps = psA.tile([P, CB], F32, tag="psA")
                    for k in range(KD):
                        nc.tensor.matmul(
                            ps[:, :cb],
                            w1_sb[k][:, m * P : (m + 1) * P],
                            x_sb[k][:, c0 : c0 + cb],
                            start=(k == 0),
                            stop=(k == KD - 1),
                        )
                    nc.scalar.activation(
                        ht[:, m, :cb], ps[:, :cb], GELU, bias=b1_sb[:, m : m + 1]
                    )
                # Layer 2: y[c, d] = gate[c] * (sum_f ht[f, c] * w2t[f, d])
                for m2 in range(cb // P):
                    tglob = c0 // P + m2
                    ps2 = psB.tile([P, D], F32, tag="psB")
                    for k in range(MF):
                        for n2 in range(ND):
                            nc.tensor.matmul(
                                ps2[:, n2 * 512 : (n2 + 1) * 512],
                                ht[:, k, m2 * P : (m2 + 1) * P],
                                w2_sb[k][:, n2 * 512 : (n2 + 1) * 512],
                                start=(k == 0),
                                stop=(k == MF - 1),
                            )
                    ysb = yp.tile([P, D], F32, tag="y")
                    nc.vector.tensor_scalar_mul(
                        ysb[:], ps2[:], gate_sb[:, tglob : tglob + 1]
                    )
                    nc.sync.dma_start(out=yv[tglob], in_=ysb[:])
                c0 += cb
    nc.compile()
    return nc


def _get_program(D: int, F: int, C: int):
    key = (D, F, C)
    if key not in _PROGRAM_CACHE:
        _PROGRAM_CACHE[key] = _build_program(D, F, C)
    return _PROGRAM_CACHE[key]


def _route(xf, Wr, top_k):
    """Host router mirroring the reference: softmax -> top-k -> renormalize."""
    T, E = xf.shape[0], Wr.shape[0]
    logits = (xf @ Wr.T).astype(np.float64)  # fp32 BLAS, then fp64 softmax
    logits -= logits.max(axis=1, keepdims=True)
    p = np.exp(logits)
    p /= p.sum(axis=1, keepdims=True)
    order = np.argsort(-p, axis=1, kind="stable")  # ties -> lower index, like jax
    topk_idx = order[:, :top_k]
    topk_p = np.take_along_axis(p, topk_idx, axis=1)
    gates = topk_p / topk_p.sum(axis=1, keepdims=True)

    usage = p.mean(axis=0)
    aux = np.float32(LB_WEIGHT * np.sum((usage - 1.0 / E) ** 2))

    gate_te = np.zeros((T, E), np.float32)
    np.put_along_axis(gate_te, topk_idx, gates.astype(np.float32), axis=1)
    sel = np.zeros((T, E), bool)
    np.put_along_axis(sel, topk_idx, True, axis=1)
    return gate_te, sel, aux


def kernel(**inputs):
    x = np.asarray(inputs["x"], np.float32)
    Wr = np.asarray(inputs["Wr"], np.float32)
    W1 = np.asarray(inputs["W1"], np.float32)
    b1 = np.asarray(inputs["b1"], np.float32)
    W2 = np.asarray(inputs["W2"], np.float32)
    b2 = np.asarray(inputs["b2"], np.float32)
    top_k = int(np.asarray(inputs["top_k"]))

    B, S, D = x.shape
    E, F = W1.shape[0], W1.shape[1]
    T = B * S
    xf = x.reshape(T, D)
    assert E <= N_CORES, "one expert per core"

    gate_te, sel, aux = _route(xf, Wr, top_k)

    idx_lists = [np.nonzero(sel[:, e])[0] for e in range(E)]
    max_count = max(1, max(len(ix) for ix in idx_lists))
    C = -(-max_count // P) * P  # pad to partition multiple

    in_maps = []
    for e in range(N_CORES):
        if e < E:
            ix = idx_lists[e]
            n = len(ix)
            xTe = np.zeros((D, C), BF16_NP)
            xTe[:, :n] = xf[ix].T.astype(BF16_NP)
            g = np.zeros((C,), np.float32)
            g[:n] = gate_te[ix, e]
            in_maps.append(
                {
                    "xT": xTe,
                    "w1t": np.ascontiguousarray(W1[e].T).astype(BF16_NP),
                    "w2t": np.ascontiguousarray(W2[e].T).astype(BF16_NP),
                    "b1": np.ascontiguousarray(b1[e].reshape(F // P, P)),
                    "gate": g.reshape(C // P, P),
                }
            )
        else:  # idle core: zero work, same program
            in_maps.append(
                {
                    "xT": np.zeros((D, C), BF16_NP),
                    "w1t": np.zeros((D, F), BF16_NP),
                    "w2t": np.zeros((F, D), BF16_NP),
                    "b1": np.zeros((F // P, P), np.float32),
                    "gate": np.zeros((C // P, P), np.float32),
                }
            )

    nc = _get_program(D, F, C)
    res = run_bass_kernel_spmd(nc, in_maps, core_ids=list(range(N_CORES)))

    out_flat = np.zeros((T, D), np.float32)
    for e in range(E):
        ix = idx_lists[e]
        if len(ix) == 0:
            continue
        out_flat[ix] += res.results[e]["y"][: len(ix)]
        if np.any(b2[e]):
            out_flat[ix] += np.outer(gate_te[ix, e], b2[e])

    return out_flat.reshape(B, S, D), aux


# revision 14
# speedup vs baseline: 1.0202x; 1.0202x over previous
"""MoE layer (softmax router, top-k, per-expert FFN D->F->gelu->D) on 8 TRN2 cores.

Strategy: expert-parallel. The router (a tiny T x E matmul + top-k) runs on the
host and doubles as the sharding function: core e receives exactly the tokens
whose top-k set contains expert e (the "all-to-all dispatch"), already
transposed to (D, C) so both device matmuls need no on-chip transposes.
Each core holds one expert's weights (pre-transposed on host, cast to bf16),
computes gelu(x @ W1^T + b1) @ W2^T, scales each token's output by its
renormalized gate on-chip, and the host scatter-adds the per-expert results
back into the full (T, D) output. b2 and the aux load-balance loss are added
on the host (b2 enters linearly: sum_e gate * b2[e]).
"""

import numpy as np
import ml_dtypes

import concourse.bass as bass  # noqa: F401  (registers AP machinery)
import concourse.mybir as mybir
import concourse.tile as tile
from concourse import bacc
from concourse.bass_utils import run_bass_kernel_spmd

P = 128
CB = 512  # token-block: free dim of layer-1 matmuls, partition chunk of layer-2
LB_WEIGHT = 0.01
N_CORES = 8

BF16 = mybir.dt.bfloat16
F32 = mybir.dt.float32
BF16_NP = ml_dtypes.bfloat16

_PROGRAM_CACHE: dict = {}


def _token_blocks(C: int) -> list[int]:
    blocks = [CB] * (C // CB)
    if C % CB:
        blocks.append(C % CB)
    return blocks


def _build_program(
    D: int,
    F: int,
    C: int,
    repeat: int = 1,
    psa_bufs: int = 3,
    psb_bufs: int = 2,
    y_bufs: int = 3,
    w2_late: bool = False,
    ht_split: int = 1,
    w1_chunk: int = 4,  # if >0, split each w1 k-tile into column chunks of this many m-tiles
):
    """One-expert FFN over C (padded) tokens; SPMD-identical across cores."""
    assert D % P == 0 and F % P == 0 and C % P == 0 and D % 512 == 0
    KD, MF, ND = D // P, F // P, D // 512
    offs, c0 = [], 0
    for cb in _token_blocks(C):
        offs.append((c0, cb))
        c0 += cb
    blocks = offs * repeat
    GELU = mybir.ActivationFunctionType.Gelu_apprx_tanh

    nc = bacc.Bacc(None, target_bir_lowering=False, debug=False)
    xT = nc.declare_dram_parameter("xT", [D, C], BF16, isOutput=False)
    w1t = nc.declare_dram_parameter("w1t", [D, F], BF16, isOutput=False)
    w2t = nc.declare_dram_parameter("w2t", [F, D], BF16, isOutput=False)
    b1 = nc.declare_dram_parameter("b1", [MF, P], F32, isOutput=False)
    gate = nc.declare_dram_parameter("gate", [C // P, P], F32, isOutput=False)
    y = nc.declare_dram_parameter("y", [C, D], F32, isOutput=True)

    w1v = w1t.rearrange("(k p) f -> k p f", p=P)
    w2v = w2t.rearrange("(k p) d -> k p d", p=P)
    xv = xT.rearrange("(k p) c -> k p c", p=P)
    yv = y.rearrange("(t p) d -> t p d", p=P)

    with tile.TileContext(nc) as tc:
        with (
            tc.tile_pool(name="const", bufs=1) as constp,
            tc.tile_pool(name="wres", bufs=1) as wres,
            tc.tile_pool(name="xres", bufs=1) as xres,
            tc.tile_pool(name="htp", bufs=1) as htp,
            tc.tile_pool(name="ysb", bufs=y_bufs) as yp,
            tc.tile_pool(name="psA", bufs=psa_bufs, space="PSUM") as psA,
            tc.tile_pool(name="psB", bufs=psb_bufs, space="PSUM") as psB,
        ):
            b1_sb = constp.tile([P, MF], F32, tag="b1")
            nc.sync.dma_start(out=b1_sb[:], in_=b1.rearrange("m p -> p m"))
            gate_sb = constp.tile([P, C // P], F32, tag="gate")
            nc.sync.dma_start(out=gate_sb[:], in_=gate.rearrange("t p -> p t"))

            x_sb = []
            for k in range(KD):
                t = xres.tile([P, C], BF16, tag=f"x{k}")
                nc.sync.dma_start(out=t[:], in_=xv[k])
                x_sb.append(t)
            if w1_chunk:
                nch = -(-MF // w1_chunk)
                w1_sb = [[None] * nch for _ in range(KD)]
                for ch in range(nch):
                    for k in range(KD):
                        mw = min(w1_chunk, MF - ch * w1_chunk) * P
                        t = wres.tile([P, mw], BF16, tag=f"w1_{k}_{ch}", name=f"w1_{k}_{ch}")
                        nc.sync.dma_start(
                            out=t[:], in_=w1v[k][:, ch * w1_chunk * P :][:, :mw]
                        )
                        w1_sb[k][ch] = t

                def w1_slice(k, m):
                    return w1_sb[k][m // w1_chunk][
                        :, (m % w1_chunk) * P : (m % w1_chunk + 1) * P
                    ]
            else:
                w1_sb = []
                for k in range(KD):
                    t = wres.tile([P, F], BF16, tag=f"w1_{k}")
                    nc.sync.dma_start(out=t[:], in_=w1v[k])
                    w1_sb.append(t)

                def w1_slice(k, m):
                    return w1_sb[k][:, m * P : (m + 1) * P]
            w2_sb = []

            def load_w2():
                for k in range(MF):
                    t = wres.tile([P, D], BF16, tag=f"w2_{k}")
                    nc.sync.dma_start(out=t[:], in_=w2v[k])
                    w2_sb.append(t)

            if not w2_late:
                load_w2()

            MS = MF // ht_split  # m-tiles per ht sub-tile
            for bi, (c0, cb) in enumerate(blocks):
                # Layer 1: hT[f, c] = gelu(sum_d w1t[d, f] * xT[d, c] + b1[f])
                hts = [
                    htp.tile([P, MS, CB], BF16, tag=f"ht{s}", name=f"ht{s}")
                    for s in range(ht_split)
                ]
                for m in range(MF):
                    ps = psA.tile([P, CB], F32, tag="psA")
                    for k in range(KD):
                        nc.tensor.matmul(
                            ps[:, :cb],
                            w1_slice(k, m),
                            x_sb[k][:, c0 : c0 + cb],
                            start=(k == 0),
                            stop=(k == KD - 1),
                        )
                    nc.scalar.activation(
                        hts[m // MS][:, m % MS, :cb],
                        ps[:, :cb],
                        GELU,
                        bias=b1_sb[:, m : m + 1],
                    )
                if bi == 0 and w2_late:
                    load_w2()
                # Layer 2: y[c, d] = gate[c] * (sum_f ht[f, c] * w2t[f, d])
                for m2 in range(cb // P):
                    tglob = c0 // P + m2
                    ps2 = psB.tile([P, D], F32, tag="psB")
                    for k in range(MF):
                        for n2 in range(ND):
                            nc.tensor.matmul(
                                ps2[:, n2 * 512 : (n2 + 1) * 512],
                                hts[k // MS][:, k % MS, m2 * P : (m2 + 1) * P],
                                w2_sb[k][:, n2 * 512 : (n2 + 1) * 512],
                                start=(k == 0),
                                stop=(k == MF - 1),
                            )
                    ysb = yp.tile([P, D], F32, tag="y")
                    nc.vector.tensor_scalar_mul(
                        ysb[:], ps2[:], gate_sb[:, tglob : tglob + 1]
                    )
                    nc.sync.dma_start(out=yv[tglob], in_=ysb[:])
    nc.compile()
    return nc


def _get_program(D: int, F: int, C: int):
    key = (D, F, C)
    if key not in _PROGRAM_CACHE:
        _PROGRAM_CACHE[key] = _build_program(D, F, C)
    return _PROGRAM_CACHE[key]


def _route(xf, Wr, top_k):
    """Host router mirroring the reference: softmax -> top-k -> renormalize."""
    T, E = xf.shape[0], Wr.shape[0]
    logits = (xf @ Wr.T).astype(np.float64)  # fp32 BLAS, then fp64 softmax
    logits -= logits.max(axis=1, keepdims=True)
    p = np.exp(logits)
    p /= p.sum(axis=1, keepdims=True)
    order = np.argsort(-p, axis=1, kind="stable")  # ties -> lower index, like jax
    topk_idx = order[:, :top_k]
    topk_p = np.take_along_axis(p, topk_idx, axis=1)
    gates = topk_p / topk_p.sum(axis=1, keepdims=True)

    usage = p.mean(axis=0)
    aux = np.float32(LB_WEIGHT * np.sum((usage - 1.0 / E) ** 2))

    gate_te = np.zeros((T, E), np.float32)
    np.put_along_axis(gate_te, topk_idx, gates.astype(np.float32), axis=1)
    sel = np.zeros((T, E), bool)
    np.put_along_axis(sel, topk_idx, True, axis=1)
    return gate_te, sel, aux


def kernel(**inputs):
    x = np.asarray(inputs["x"], np.float32)
    Wr = np.asarray(inputs["Wr"], np.float32)
    W1 = np.asarray(inputs["W1"], np.float32)
    b1 = np.asarray(inputs["b1"], np.float32)
    W2 = np.asarray(inputs["W2"], np.float32)
    b2 = np.asarray(inputs["b2"], np.float32)
    top_k = int(np.asarray(inputs["top_k"]))

    B, S, D = x.shape
    E, F = W1.shape[0], W1.shape[1]
    T = B * S
    xf = x.reshape(T, D)
    assert E <= N_CORES, "one expert per core"

    gate_te, sel, aux = _route(xf, Wr, top_k)

    idx_lists = [np.nonzero(sel[:, e])[0] for e in range(E)]
    max_count = max(1, max(len(ix) for ix in idx_lists))
    C = -(-max_count // P) * P  # pad to partition multiple

    in_maps = []
    for e in range(N_CORES):
        if e < E:
            ix = idx_lists[e]
            n = len(ix)
            xTe = np.zeros((D, C), BF16_NP)
            xTe[:, :n] = xf[ix].T.astype(BF16_NP)
            g = np.zeros((C,), np.float32)
            g[:n] = gate_te[ix, e]
            in_maps.append(
                {
                    "xT": xTe,
                    "w1t": np.ascontiguousarray(W1[e].T).astype(BF16_NP),
                    "w2t": np.ascontiguousarray(W2[e].T).astype(BF16_NP),
                    "b1": np.ascontiguousarray(b1[e].reshape(F // P, P)),
                    "gate": g.reshape(C // P, P),
                }
            )
        else:  # idle core: zero work, same program
            in_maps.append(
                {
                    "xT": np.zeros((D, C), BF16_NP),
                    "w1t": np.zeros((D, F), BF16_NP),
                    "w2t": np.zeros((F, D), BF16_NP),
                    "b1": np.zeros((F // P, P), np.float32),
                    "gate": np.zeros((C // P, P), np.float32),
                }
            )

    nc = _get_program(D, F, C)
    res = run_bass_kernel_spmd(nc, in_maps, core_ids=list(range(N_CORES)))

    out_flat = np.zeros((T, D), np.float32)
    for e in range(E):
        ix = idx_lists[e]
        if len(ix) == 0:
            continue
        out_flat[ix] += res.results[e]["y"][: len(ix)]
        if np.any(b2[e]):
            out_flat[ix] += np.outer(gate_te[ix, e], b2[e])

    return out_flat.reshape(B, S, D), aux


# revision 24
# speedup vs baseline: 290.4584x; 284.6989x over previous
"""MoE layer (softmax router, top-k, per-expert FFN D->F->gelu->D) on 8 TRN2 cores.

Strategy: expert-parallel. The router (a tiny T x E matmul + top-k) runs on the
host and doubles as the sharding function: core e receives exactly the tokens
whose top-k set contains expert e (the "all-to-all dispatch"), already
transposed to (D, C) so both device matmuls need no on-chip transposes.
Each core holds one expert's weights (pre-transposed on host, cast to bf16),
computes gelu(x @ W1^T + b1) @ W2^T, scales each token's output by its
renormalized gate on-chip, and the host scatter-adds the per-expert results
back into the full (T, D) output. b2 and the aux load-balance loss are added
on the host (b2 enters linearly: sum_e gate * b2[e]).
"""

import numpy as np
import ml_dtypes

import concourse.bass as bass  # noqa: F401  (registers AP machinery)
import concourse.mybir as mybir
import concourse.tile as tile
from concourse import bacc
from concourse.bass_utils import run_bass_kernel_spmd

P = 128
CB = 512  # token-block: free dim of layer-1 matmuls, partition chunk of layer-2
LB_WEIGHT = 0.01
N_CORES = 8

BF16 = mybir.dt.bfloat16
F32 = mybir.dt.float32
BF16_NP = ml_dtypes.bfloat16

_PROGRAM_CACHE: dict = {}


def _token_blocks(C: int) -> list[int]:
    """Split C into ceil(C/CB) near-equal blocks, each a multiple of P.

    Balanced blocks keep every layer-1 matmul's free dim large (>=256 for any
    C >= 512) instead of leaving a 128-wide remainder block whose matmuls pay
    proportionally more issue/LDWEIGHTS overhead on hardware.
    """
    nb = -(-C // CB)
    base = C // nb // P * P
    blocks = [base] * nb
    extra = (C - base * nb) // P
    for i in range(extra):
        blocks[i] += P
    return blocks


def _build_program(
    D: int,
    F: int,
    C: int,
    repeat: int = 1,
    psa_bufs: int = 3,
    psb_bufs: int = 2,
    y_bufs: int = 3,
    w2_late: bool = False,
    ht_split: int = 1,
    w1_chunk: int = 4,  # if >0, split each w1 k-tile into column chunks of this many m-tiles
    small_first: bool = False,  # schedule the remainder block first (startup is DMA-bound)
):
    """One-expert FFN over C (padded) tokens; SPMD-identical across cores."""
    assert D % P == 0 and F % P == 0 and C % P == 0 and D % 512 == 0
    KD, MF, ND = D // P, F // P, D // 512
    offs, c0 = [], 0
    for cb in _token_blocks(C):
        offs.append((c0, cb))
        c0 += cb
    if small_first:
        offs.sort(key=lambda t: t[1])
    blocks = offs * repeat
    GELU = mybir.ActivationFunctionType.Gelu_apprx_tanh

    nc = bacc.Bacc(None, target_bir_lowering=False, debug=False)
    xT = nc.declare_dram_parameter("xT", [D, C], BF16, isOutput=False)
    w1t = nc.declare_dram_parameter("w1t", [D, F], BF16, isOutput=False)
    w2t = nc.declare_dram_parameter("w2t", [F, D], BF16, isOutput=False)
    b1 = nc.declare_dram_parameter("b1", [MF, P], F32, isOutput=False)
    gate = nc.declare_dram_parameter("gate", [C // P, P], F32, isOutput=False)
    y = nc.declare_dram_parameter("y", [C, D], F32, isOutput=True)

    w1v = w1t.rearrange("(k p) f -> k p f", p=P)
    w2v = w2t.rearrange("(k p) d -> k p d", p=P)
    xv = xT.rearrange("(k p) c -> k p c", p=P)
    yv = y.rearrange("(t p) d -> t p d", p=P)

    with tile.TileContext(nc) as tc:
        with (
            tc.tile_pool(name="const", bufs=1) as constp,
            tc.tile_pool(name="wres", bufs=1) as wres,
            tc.tile_pool(name="xres", bufs=1) as xres,
            tc.tile_pool(name="htp", bufs=1) as htp,
            tc.tile_pool(name="ysb", bufs=y_bufs) as yp,
            tc.tile_pool(name="psA", bufs=psa_bufs, space="PSUM") as psA,
            tc.tile_pool(name="psB", bufs=psb_bufs, space="PSUM") as psB,
        ):
            b1_sb = constp.tile([P, MF], F32, tag="b1")
            nc.sync.dma_start(out=b1_sb[:], in_=b1.rearrange("m p -> p m"))
            gate_sb = constp.tile([P, C // P], F32, tag="gate")
            nc.sync.dma_start(out=gate_sb[:], in_=gate.rearrange("t p -> p t"))

            x_sb = []
            for k in range(KD):
                t = xres.tile([P, C], BF16, tag=f"x{k}")
                nc.sync.dma_start(out=t[:], in_=xv[k])
                x_sb.append(t)
            if w1_chunk:
                nch = -(-MF // w1_chunk)
                w1_sb = [[None] * nch for _ in range(KD)]
                for ch in range(nch):
                    for k in range(KD):
                        mw = min(w1_chunk, MF - ch * w1_chunk) * P
                        t = wres.tile([P, mw], BF16, tag=f"w1_{k}_{ch}", name=f"w1_{k}_{ch}")
                        nc.sync.dma_start(
                            out=t[:], in_=w1v[k][:, ch * w1_chunk * P :][:, :mw]
                        )
                        w1_sb[k][ch] = t

                def w1_slice(k, m):
                    return w1_sb[k][m // w1_chunk][
                        :, (m % w1_chunk) * P : (m % w1_chunk + 1) * P
                    ]
            else:
                w1_sb = []
                for k in range(KD):
                    t = wres.tile([P, F], BF16, tag=f"w1_{k}")
                    nc.sync.dma_start(out=t[:], in_=w1v[k])
                    w1_sb.append(t)

                def w1_slice(k, m):
                    return w1_sb[k][:, m * P : (m + 1) * P]
            w2_sb = []

            def load_w2():
                for k in range(MF):
                    t = wres.tile([P, D], BF16, tag=f"w2_{k}")
                    nc.sync.dma_start(out=t[:], in_=w2v[k])
                    w2_sb.append(t)

            if not w2_late:
                load_w2()

            MS = MF // ht_split  # m-tiles per ht sub-tile
            CBMAX = max(cb for _, cb in blocks)
            for bi, (c0, cb) in enumerate(blocks):
                # Layer 1: hT[f, c] = gelu(sum_d w1t[d, f] * xT[d, c] + b1[f])
                hts = [
                    htp.tile([P, MS, CBMAX], BF16, tag=f"ht{s}", name=f"ht{s}")
                    for s in range(ht_split)
                ]
                for m in range(MF):
                    ps = psA.tile([P, CBMAX], F32, tag="psA")
                    for k in range(KD):
                        nc.tensor.matmul(
                            ps[:, :cb],
                            w1_slice(k, m),
                            x_sb[k][:, c0 : c0 + cb],
                            start=(k == 0),
                            stop=(k == KD - 1),
                        )
                    nc.scalar.activation(
                        hts[m // MS][:, m % MS, :cb],
                        ps[:, :cb],
                        GELU,
                        bias=b1_sb[:, m : m + 1],
                    )
                if bi == 0 and w2_late:
                    load_w2()
                # Layer 2: y[c, d] = gate[c] * (sum_f ht[f, c] * w2t[f, d])
                for m2 in range(cb // P):
                    tglob = c0 // P + m2
                    ps2 = psB.tile([P, D], F32, tag="psB")
                    for k in range(MF):
                        for n2 in range(ND):
                            nc.tensor.matmul(
                                ps2[:, n2 * 512 : (n2 + 1) * 512],
                                hts[k // MS][:, k % MS, m2 * P : (m2 + 1) * P],
                                w2_sb[k][:, n2 * 512 : (n2 + 1) * 512],
                                start=(k == 0),
                                stop=(k == MF - 1),
                            )
                    ysb = yp.tile([P, D], F32, tag="y")
                    nc.vector.tensor_scalar_mul(
                        ysb[:], ps2[:], gate_sb[:, tglob : tglob + 1]
                    )
                    nc.sync.dma_start(out=yv[tglob], in_=ysb[:])
    nc.compile()
    return nc


def _get_program(D: int, F: int, C: int):
    key = (D, F, C)
    if key not in _PROGRAM_CACHE:
        _PROGRAM_CACHE[key] = _build_program(D, F, C)
    return _PROGRAM_CACHE[key]


def _route(xf, Wr, top_k):
    """Host router mirroring the reference: softmax -> top-k -> renormalize."""
    T, E = xf.shape[0], Wr.shape[0]
    logits = (xf @ Wr.T).astype(np.float64)  # fp32 BLAS, then fp64 softmax
    logits -= logits.max(axis=1, keepdims=True)
    p = np.exp(logits)
    p /= p.sum(axis=1, keepdims=True)
    order = np.argsort(-p, axis=1, kind="stable")  # ties -> lower index, like jax
    topk_idx = order[:, :top_k]
    topk_p = np.take_along_axis(p, topk_idx, axis=1)
    gates = topk_p / topk_p.sum(axis=1, keepdims=True)

    usage = p.mean(axis=0)
    aux = np.float32(LB_WEIGHT * np.sum((usage - 1.0 / E) ** 2))

    gate_te = np.zeros((T, E), np.float32)
    np.put_along_axis(gate_te, topk_idx, gates.astype(np.float32), axis=1)
    sel = np.zeros((T, E), bool)
    np.put_along_axis(sel, topk_idx, True, axis=1)
    return gate_te, sel, aux


def kernel(**inputs):
    x = np.asarray(inputs["x"], np.float32)
    Wr = np.asarray(inputs["Wr"], np.float32)
    W1 = np.asarray(inputs["W1"], np.float32)
    b1 = np.asarray(inputs["b1"], np.float32)
    W2 = np.asarray(inputs["W2"], np.float32)
    b2 = np.asarray(inputs["b2"], np.float32)
    top_k = int(np.asarray(inputs["top_k"]))

    B, S, D = x.shape
    E, F = W1.shape[0], W1.shape[1]
    T = B * S
    xf = x.reshape(T, D)
    assert E <= N_CORES, "one expert per core"

    gate_te, sel, aux = _route(xf, Wr, top_k)

    idx_lists = [np.nonzero(sel[:, e])[0] for e in range(E)]
    max_count = max(1, max(len(ix) for ix in idx_lists))
    C = -(-max_count // P) * P  # pad to partition multiple

    in_maps = []
    for e in range(N_CORES):
        if e < E:
            ix = idx_lists[e]
            n = len(ix)
            xTe = np.zeros((D, C), BF16_NP)
            xTe[:, :n] = xf[ix].T.astype(BF16_NP)
            g = np.zeros((C,), np.float32)
            g[:n] = gate_te[ix, e]
            in_maps.append(
                {
                    "xT": xTe,
                    "w1t": np.ascontiguousarray(W1[e].T).astype(BF16_NP),
                    "w2t": np.ascontiguousarray(W2[e].T).astype(BF16_NP),
                    "b1": np.ascontiguousarray(b1[e].reshape(F // P, P)),
                    "gate": g.reshape(C // P, P),
                }
            )
        else:  # idle core: zero work, same program
            in_maps.append(
                {
                    "xT": np.zeros((D, C), BF16_NP),
                    "w1t": np.zeros((D, F), BF16_NP),
                    "w2t": np.zeros((F, D), BF16_NP),
                    "b1": np.zeros((F // P, P), np.float32),
                    "gate": np.zeros((C // P, P), np.float32),
                }
            )

    nc = _get_program(D, F, C)
    try:
        res = run_bass_kernel_spmd(nc, in_maps, core_ids=list(range(N_CORES)))
    except Exception:  # transient device/tunnel hiccup: one retry
        res = run_bass_kernel_spmd(nc, in_maps, core_ids=list(range(N_CORES)))

    out_flat = np.zeros((T, D), np.float32)
    for e in range(E):
        ix = idx_lists[e]
        if len(ix) == 0:
            continue
        out_flat[ix] += res.results[e]["y"][: len(ix)]
        if np.any(b2[e]):
            out_flat[ix] += np.outer(gate_te[ix, e], b2[e])

    return out_flat.reshape(B, S, D), aux


# revision 26
# speedup vs baseline: 295.5694x; 1.0176x over previous
"""MoE layer (softmax router, top-k, per-expert FFN D->F->gelu->D) on 8 TRN2 cores.

Strategy: expert-parallel. The router (a tiny T x E matmul + top-k) runs on the
host and doubles as the sharding function: core e receives exactly the tokens
whose top-k set contains expert e (the "all-to-all dispatch"), already
transposed to (D, C) so both device matmuls need no on-chip transposes.
Each core holds one expert's weights (pre-transposed on host, cast to bf16),
computes gelu(x @ W1^T + b1) @ W2^T, scales each token's output by its
renormalized gate on-chip, and the host scatter-adds the per-expert results
back into the full (T, D) output. b2 and the aux load-balance loss are added
on the host (b2 enters linearly: sum_e gate * b2[e]).
"""

import numpy as np
import ml_dtypes

import concourse.bass as bass  # noqa: F401  (registers AP machinery)
import concourse.mybir as mybir
import concourse.tile as tile
from concourse import bacc
from concourse.bass_utils import run_bass_kernel_spmd

P = 128
CB = 512  # token-block: free dim of layer-1 matmuls, partition chunk of layer-2
LB_WEIGHT = 0.01
N_CORES = 8

BF16 = mybir.dt.bfloat16
F32 = mybir.dt.float32
BF16_NP = ml_dtypes.bfloat16

_PROGRAM_CACHE: dict = {}


def _token_blocks(C: int) -> list[int]:
    """Split C into ceil(C/CB) near-equal blocks, each a multiple of P.

    Balanced blocks keep every layer-1 matmul's free dim large (>=256 for any
    C >= 512) instead of leaving a 128-wide remainder block whose matmuls pay
    proportionally more issue/LDWEIGHTS overhead on hardware.
    """
    nb = -(-C // CB)
    base = C // nb // P * P
    blocks = [base] * nb
    extra = (C - base * nb) // P
    for i in range(extra):
        blocks[i] += P
    return blocks


def _build_program(
    D: int,
    F: int,
    C: int,
    repeat: int = 1,
    psa_bufs: int = 3,
    psb_bufs: int = 2,
    y_bufs: int = 3,
    w2_late: bool = False,
    ht_split: int = 1,
    w1_chunk: int = 5,  # if >0, split each w1 k-tile into column chunks of this many m-tiles
    small_first: bool = False,  # schedule the remainder block first (startup is DMA-bound)
):
    """One-expert FFN over C (padded) tokens; SPMD-identical across cores."""
    assert D % P == 0 and F % P == 0 and C % P == 0 and D % 512 == 0
    KD, MF, ND = D // P, F // P, D // 512
    offs, c0 = [], 0
    for cb in _token_blocks(C):
        offs.append((c0, cb))
        c0 += cb
    if small_first:
        offs.sort(key=lambda t: t[1])
    blocks = offs * repeat
    GELU = mybir.ActivationFunctionType.Gelu_apprx_tanh

    nc = bacc.Bacc(None, target_bir_lowering=False, debug=False)
    xT = nc.declare_dram_parameter("xT", [D, C], BF16, isOutput=False)
    w1t = nc.declare_dram_parameter("w1t", [D, F], BF16, isOutput=False)
    w2t = nc.declare_dram_parameter("w2t", [F, D], BF16, isOutput=False)
    b1 = nc.declare_dram_parameter("b1", [MF, P], F32, isOutput=False)
    gate = nc.declare_dram_parameter("gate", [C // P, P], F32, isOutput=False)
    y = nc.declare_dram_parameter("y", [C, D], F32, isOutput=True)

    w1v = w1t.rearrange("(k p) f -> k p f", p=P)
    w2v = w2t.rearrange("(k p) d -> k p d", p=P)
    xv = xT.rearrange("(k p) c -> k p c", p=P)
    yv = y.rearrange("(t p) d -> t p d", p=P)

    with tile.TileContext(nc) as tc:
        with (
            tc.tile_pool(name="const", bufs=1) as constp,
            tc.tile_pool(name="wres", bufs=1) as wres,
            tc.tile_pool(name="xres", bufs=1) as xres,
            tc.tile_pool(name="htp", bufs=1) as htp,
            tc.tile_pool(name="ysb", bufs=y_bufs) as yp,
            tc.tile_pool(name="psA", bufs=psa_bufs, space="PSUM") as psA,
            tc.tile_pool(name="psB", bufs=psb_bufs, space="PSUM") as psB,
        ):
            b1_sb = constp.tile([P, MF], F32, tag="b1")
            nc.sync.dma_start(out=b1_sb[:], in_=b1.rearrange("m p -> p m"))
            gate_sb = constp.tile([P, C // P], F32, tag="gate")
            nc.sync.dma_start(out=gate_sb[:], in_=gate.rearrange("t p -> p t"))

            x_sb = []
            for k in range(KD):
                t = xres.tile([P, C], BF16, tag=f"x{k}")
                nc.sync.dma_start(out=t[:], in_=xv[k])
                x_sb.append(t)
            if w1_chunk:
                nch = -(-MF // w1_chunk)
                w1_sb = [[None] * nch for _ in range(KD)]
                for ch in range(nch):
                    for k in range(KD):
                        mw = min(w1_chunk, MF - ch * w1_chunk) * P
                        t = wres.tile([P, mw], BF16, tag=f"w1_{k}_{ch}", name=f"w1_{k}_{ch}")
                        nc.sync.dma_start(
                            out=t[:], in_=w1v[k][:, ch * w1_chunk * P :][:, :mw]
                        )
                        w1_sb[k][ch] = t

                def w1_slice(k, m):
                    return w1_sb[k][m // w1_chunk][
                        :, (m % w1_chunk) * P : (m % w1_chunk + 1) * P
                    ]
            else:
                w1_sb = []
                for k in range(KD):
                    t = wres.tile([P, F], BF16, tag=f"w1_{k}")
                    nc.sync.dma_start(out=t[:], in_=w1v[k])
                    w1_sb.append(t)

                def w1_slice(k, m):
                    return w1_sb[k][:, m * P : (m + 1) * P]
            w2_sb = []

            def load_w2():
                for k in range(MF):
                    t = wres.tile([P, D], BF16, tag=f"w2_{k}")
                    nc.sync.dma_start(out=t[:], in_=w2v[k])
                    w2_sb.append(t)

            if not w2_late:
                load_w2()

            MS = MF // ht_split  # m-tiles per ht sub-tile
            CBMAX = max(cb for _, cb in blocks)
            for bi, (c0, cb) in enumerate(blocks):
                # Layer 1: hT[f, c] = gelu(sum_d w1t[d, f] * xT[d, c] + b1[f])
                hts = [
                    htp.tile([P, MS, CBMAX], BF16, tag=f"ht{s}", name=f"ht{s}")
                    for s in range(ht_split)
                ]
                for m in range(MF):
                    ps = psA.tile([P, CBMAX], F32, tag="psA")
                    for k in range(KD):
                        nc.tensor.matmul(
                            ps[:, :cb],
                            w1_slice(k, m),
                            x_sb[k][:, c0 : c0 + cb],
                            start=(k == 0),
                            stop=(k == KD - 1),
                        )
                    nc.scalar.activation(
                        hts[m // MS][:, m % MS, :cb],
                        ps[:, :cb],
                        GELU,
                        bias=b1_sb[:, m : m + 1],
                    )
                if bi == 0 and w2_late:
                    load_w2()
                # Layer 2: y[c, d] = gate[c] * (sum_f ht[f, c] * w2t[f, d])
                for m2 in range(cb // P):
                    tglob = c0 // P + m2
                    ps2 = psB.tile([P, D], F32, tag="psB")
                    for k in range(MF):
                        for n2 in range(ND):
                            nc.tensor.matmul(
                                ps2[:, n2 * 512 : (n2 + 1) * 512],
                                hts[k // MS][:, k % MS, m2 * P : (m2 + 1) * P],
                                w2_sb[k][:, n2 * 512 : (n2 + 1) * 512],
                                start=(k == 0),
                                stop=(k == MF - 1),
                            )
                    ysb = yp.tile([P, D], F32, tag="y")
                    nc.vector.tensor_scalar_mul(
                        ysb[:], ps2[:], gate_sb[:, tglob : tglob + 1]
                    )
                    nc.sync.dma_start(out=yv[tglob], in_=ysb[:])
    nc.compile()
    return nc


def _get_program(D: int, F: int, C: int):
    key = (D, F, C)
    if key not in _PROGRAM_CACHE:
        _PROGRAM_CACHE[key] = _build_program(D, F, C)
    return _PROGRAM_CACHE[key]


def _route(xf, Wr, top_k):
    """Host router mirroring the reference: softmax -> top-k -> renormalize."""
    T, E = xf.shape[0], Wr.shape[0]
    logits = (xf @ Wr.T).astype(np.float64)  # fp32 BLAS, then fp64 softmax
    logits -= logits.max(axis=1, keepdims=True)
    p = np.exp(logits)
    p /= p.sum(axis=1, keepdims=True)
    order = np.argsort(-p, axis=1, kind="stable")  # ties -> lower index, like jax
    topk_idx = order[:, :top_k]
    topk_p = np.take_along_axis(p, topk_idx, axis=1)
    gates = topk_p / topk_p.sum(axis=1, keepdims=True)

    usage = p.mean(axis=0)
    aux = np.float32(LB_WEIGHT * np.sum((usage - 1.0 / E) ** 2))

    gate_te = np.zeros((T, E), np.float32)
    np.put_along_axis(gate_te, topk_idx, gates.astype(np.float32), axis=1)
    sel = np.zeros((T, E), bool)
    np.put_along_axis(sel, topk_idx, True, axis=1)
    return gate_te, sel, aux


def kernel(**inputs):
    x = np.asarray(inputs["x"], np.float32)
    Wr = np.asarray(inputs["Wr"], np.float32)
    W1 = np.asarray(inputs["W1"], np.float32)
    b1 = np.asarray(inputs["b1"], np.float32)
    W2 = np.asarray(inputs["W2"], np.float32)
    b2 = np.asarray(inputs["b2"], np.float32)
    top_k = int(np.asarray(inputs["top_k"]))

    B, S, D = x.shape
    E, F = W1.shape[0], W1.shape[1]
    T = B * S
    xf = x.reshape(T, D)
    assert E <= N_CORES, "one expert per core"

    gate_te, sel, aux = _route(xf, Wr, top_k)

    idx_lists = [np.nonzero(sel[:, e])[0] for e in range(E)]
    max_count = max(1, max(len(ix) for ix in idx_lists))
    # Per-round token capacity. MAX_C bounds SBUF residency (the xT tile is
    # C*2 bytes/partition); heavier imbalance just takes extra rounds.
    MAX_C = 1792
    rounds = -(-max_count // MAX_C)
    per_round = -(-max_count // rounds)
    C = min(-(-per_round // P) * P, MAX_C)

    w1t_maps = [np.ascontiguousarray(W1[e].T).astype(BF16_NP) for e in range(E)]
    w2t_maps = [np.ascontiguousarray(W2[e].T).astype(BF16_NP) for e in range(E)]
    zeros = {
        "xT": np.zeros((D, C), BF16_NP),
        "w1t": np.zeros((D, F), BF16_NP),
        "w2t": np.zeros((F, D), BF16_NP),
        "b1": np.zeros((F // P, P), np.float32),
        "gate": np.zeros((C // P, P), np.float32),
    }

    nc = _get_program(D, F, C)
    out_flat = np.zeros((T, D), np.float32)
    for r in range(rounds):
        in_maps = []
        for e in range(N_CORES):
            ix = idx_lists[e][r * C : (r + 1) * C] if e < E else []
            if len(ix) == 0:
                in_maps.append(zeros)
                continue
            n = len(ix)
            xTe = np.zeros((D, C), BF16_NP)
            xTe[:, :n] = xf[ix].T.astype(BF16_NP)
            g = np.zeros((C,), np.float32)
            g[:n] = gate_te[ix, e]
            in_maps.append(
                {
                    "xT": xTe,
                    "w1t": w1t_maps[e],
                    "w2t": w2t_maps[e],
                    "b1": np.ascontiguousarray(b1[e].reshape(F // P, P)),
                    "gate": g.reshape(C // P, P),
                }
            )
        try:
            res = run_bass_kernel_spmd(nc, in_maps, core_ids=list(range(N_CORES)))
        except Exception:  # transient device/tunnel hiccup: one retry
            res = run_bass_kernel_spmd(nc, in_maps, core_ids=list(range(N_CORES)))
        for e in range(E):
            ix = idx_lists[e][r * C : (r + 1) * C]
            if len(ix) == 0:
                continue
            out_flat[ix] += res.results[e]["y"][: len(ix)]

    for e in range(E):
        ix = idx_lists[e]
        if len(ix) and np.any(b2[e]):
            out_flat[ix] += np.outer(gate_te[ix, e], b2[e])

    return out_flat.reshape(B, S, D), aux


# revision 33
# speedup vs baseline: 304.6499x; 1.0307x over previous
"""MoE layer (softmax router, top-k, per-expert FFN D->F->gelu->D) on 8 TRN2 cores.

Strategy: expert-parallel. The router (a tiny T x E matmul + top-k) runs on the
host and doubles as the sharding function: core e receives exactly the tokens
whose top-k set contains expert e (the "all-to-all dispatch"), already
transposed to (D, C) so both device matmuls need no on-chip transposes.
Each core holds one expert's weights (pre-transposed on host, cast to bf16),
computes gelu(x @ W1^T + b1) @ W2^T, scales each token's output by its
renormalized gate on-chip, and the host scatter-adds the per-expert results
back into the full (T, D) output. b2 and the aux load-balance loss are added
on the host (b2 enters linearly: sum_e gate * b2[e]).
"""

import numpy as np
import ml_dtypes

import concourse.bass as bass  # noqa: F401  (registers AP machinery)
import concourse.mybir as mybir
import concourse.tile as tile
from concourse import bacc
from concourse.bass_utils import run_bass_kernel_spmd

P = 128
CB = 512  # token-block: free dim of layer-1 matmuls, partition chunk of layer-2
LB_WEIGHT = 0.01
N_CORES = 8

BF16 = mybir.dt.bfloat16
F32 = mybir.dt.float32
BF16_NP = ml_dtypes.bfloat16

_PROGRAM_CACHE: dict = {}


def _token_blocks(C: int) -> list[int]:
    """Split C into ceil(C/CB) near-equal blocks, each a multiple of P.

    Balanced blocks keep every layer-1 matmul's free dim large (>=256 for any
    C >= 512) instead of leaving a 128-wide remainder block whose matmuls pay
    proportionally more issue/LDWEIGHTS overhead on hardware.
    """
    nb = -(-C // CB)
    base = C // nb // P * P
    blocks = [base] * nb
    extra = (C - base * nb) // P
    for i in range(extra):
        blocks[i] += P
    return blocks


def _build_program(
    D: int,
    F: int,
    C: int,
    ceff: int | None = None,  # true token count; layer 1 computes only these columns
    repeat: int = 1,
    psa_bufs: int = 3,
    psb_bufs: int = 2,
    y_bufs: int = 3,
    w2_late: bool = False,
    ht_split: int = 1,
    w1_chunk: int = 5,  # if >0, split each w1 k-tile into column chunks of this many m-tiles
    small_first: bool = False,  # schedule the remainder block first (startup is DMA-bound)
):
    """One-expert FFN over C (padded) tokens; SPMD-identical across cores.

    ceff < C skips layer-1 compute for the pad columns: layer 2 then reads
    uninitialized hT columns for tokens >= ceff, producing garbage only in
    output rows >= ceff, which carry gate 0 and are discarded by the host.
    """
    if ceff is None:
        ceff = C
    assert D % P == 0 and F % P == 0 and C % P == 0 and D % 512 == 0
    assert 0 < ceff <= C
    KD, MF, ND = D // P, F // P, D // 512
    offs, c0 = [], 0
    for cb in _token_blocks(C):
        offs.append((c0, cb))
        c0 += cb
    if small_first:
        offs.sort(key=lambda t: t[1])
    blocks = offs * repeat
    GELU = mybir.ActivationFunctionType.Gelu_apprx_tanh

    nc = bacc.Bacc(None, target_bir_lowering=False, debug=False)
    xT = nc.declare_dram_parameter("xT", [D, C], BF16, isOutput=False)
    w1t = nc.declare_dram_parameter("w1t", [D, F], BF16, isOutput=False)
    w2t = nc.declare_dram_parameter("w2t", [F, D], BF16, isOutput=False)
    b1 = nc.declare_dram_parameter("b1", [MF, P], F32, isOutput=False)
    gate = nc.declare_dram_parameter("gate", [C // P, P], F32, isOutput=False)
    y = nc.declare_dram_parameter("y", [C, D], F32, isOutput=True)

    w1v = w1t.rearrange("(k p) f -> k p f", p=P)
    w2v = w2t.rearrange("(k p) d -> k p d", p=P)
    xv = xT.rearrange("(k p) c -> k p c", p=P)
    yv = y.rearrange("(t p) d -> t p d", p=P)

    with tile.TileContext(nc) as tc:
        with (
            tc.tile_pool(name="const", bufs=1) as constp,
            tc.tile_pool(name="wres", bufs=1) as wres,
            tc.tile_pool(name="xres", bufs=1) as xres,
            tc.tile_pool(name="htp", bufs=1) as htp,
            tc.tile_pool(name="ysb", bufs=y_bufs) as yp,
            tc.tile_pool(name="psA", bufs=psa_bufs, space="PSUM") as psA,
            tc.tile_pool(name="psB", bufs=psb_bufs, space="PSUM") as psB,
        ):
            b1_sb = constp.tile([P, MF], F32, tag="b1")
            nc.sync.dma_start(out=b1_sb[:], in_=b1.rearrange("m p -> p m"))
            gate_sb = constp.tile([P, C // P], F32, tag="gate")
            nc.sync.dma_start(out=gate_sb[:], in_=gate.rearrange("t p -> p t"))

            x_sb = []
            for k in range(KD):
                t = xres.tile([P, C], BF16, tag=f"x{k}")
                nc.sync.dma_start(out=t[:], in_=xv[k])
                x_sb.append(t)
            if w1_chunk:
                nch = -(-MF // w1_chunk)
                w1_sb = [[None] * nch for _ in range(KD)]
                for ch in range(nch):
                    for k in range(KD):
                        mw = min(w1_chunk, MF - ch * w1_chunk) * P
                        t = wres.tile([P, mw], BF16, tag=f"w1_{k}_{ch}", name=f"w1_{k}_{ch}")
                        nc.sync.dma_start(
                            out=t[:], in_=w1v[k][:, ch * w1_chunk * P :][:, :mw]
                        )
                        w1_sb[k][ch] = t

                def w1_slice(k, m):
                    return w1_sb[k][m // w1_chunk][
                        :, (m % w1_chunk) * P : (m % w1_chunk + 1) * P
                    ]
            else:
                w1_sb = []
                for k in range(KD):
                    t = wres.tile([P, F], BF16, tag=f"w1_{k}")
                    nc.sync.dma_start(out=t[:], in_=w1v[k])
                    w1_sb.append(t)

                def w1_slice(k, m):
                    return w1_sb[k][:, m * P : (m + 1) * P]
            w2_sb = []

            def load_w2():
                for k in range(MF):
                    t = wres.tile([P, D], BF16, tag=f"w2_{k}")
                    nc.sync.dma_start(out=t[:], in_=w2v[k])
                    w2_sb.append(t)

            if not w2_late:
                load_w2()

            MS = MF // ht_split  # m-tiles per ht sub-tile
            CBMAX = max(cb for _, cb in blocks)
            for bi, (c0, cb) in enumerate(blocks):
                # Layer 1: hT[f, c] = gelu(sum_d w1t[d, f] * xT[d, c] + b1[f])
                # Only the first ce (true-token) columns of the block are computed.
                ce = min(cb, max(ceff - c0, 1))
                hts = [
                    htp.tile([P, MS, CBMAX], BF16, tag=f"ht{s}", name=f"ht{s}")
                    for s in range(ht_split)
                ]
                for m in range(MF):
                    ps = psA.tile([P, CBMAX], F32, tag="psA")
                    for k in range(KD):
                        nc.tensor.matmul(
                            ps[:, :ce],
                            w1_slice(k, m),
                            x_sb[k][:, c0 : c0 + ce],
                            start=(k == 0),
                            stop=(k == KD - 1),
                        )
                    nc.scalar.activation(
                        hts[m // MS][:, m % MS, :ce],
                        ps[:, :ce],
                        GELU,
                        bias=b1_sb[:, m : m + 1],
                    )
                if bi == 0 and w2_late:
                    load_w2()
                # Layer 2: y[c, d] = gate[c] * (sum_f ht[f, c] * w2t[f, d])
                for m2 in range(cb // P):
                    tglob = c0 // P + m2
                    ps2 = psB.tile([P, D], F32, tag="psB")
                    for k in range(MF):
                        for n2 in range(ND):
                            nc.tensor.matmul(
                                ps2[:, n2 * 512 : (n2 + 1) * 512],
                                hts[k // MS][:, k % MS, m2 * P : (m2 + 1) * P],
                                w2_sb[k][:, n2 * 512 : (n2 + 1) * 512],
                                start=(k == 0),
                                stop=(k == MF - 1),
                            )
                    ysb = yp.tile([P, D], F32, tag="y")
                    nc.vector.tensor_scalar_mul(
                        ysb[:], ps2[:], gate_sb[:, tglob : tglob + 1]
                    )
                    nc.sync.dma_start(out=yv[tglob], in_=ysb[:])
    nc.compile()
    return nc


def _get_program(D: int, F: int, C: int, ceff: int):
    key = (D, F, C, ceff)
    if key not in _PROGRAM_CACHE:
        _PROGRAM_CACHE[key] = _build_program(D, F, C, ceff=ceff)
    return _PROGRAM_CACHE[key]


def _run_with_retry(nc, in_maps, attempts: int = 3):
    """Run the SPMD kernel; on a poisoned device session (e.g. transient
    NRT_EXEC_UNIT_UNRECOVERABLE through the axon tunnel) reset the PJRT
    backend — the in-process equivalent of a fresh session — and retry."""
    import time as _time

    for attempt in range(attempts):
        try:
            return run_bass_kernel_spmd(nc, in_maps, core_ids=list(range(N_CORES)))
        except Exception:
            if attempt == attempts - 1:
                raise
            try:
                import jax

                jax.clear_backends()
            except Exception:
                pass
            _time.sleep(3.0)


def _route(xf, Wr, top_k):
    """Host router mirroring the reference: softmax -> top-k -> renormalize."""
    T, E = xf.shape[0], Wr.shape[0]
    logits = (xf @ Wr.T).astype(np.float64)  # fp32 BLAS, then fp64 softmax
    logits -= logits.max(axis=1, keepdims=True)
    p = np.exp(logits)
    p /= p.sum(axis=1, keepdims=True)
    order = np.argsort(-p, axis=1, kind="stable")  # ties -> lower index, like jax
    topk_idx = order[:, :top_k]
    topk_p = np.take_along_axis(p, topk_idx, axis=1)
    gates = topk_p / topk_p.sum(axis=1, keepdims=True)

    usage = p.mean(axis=0)
    aux = np.float32(LB_WEIGHT * np.sum((usage - 1.0 / E) ** 2))

    gate_te = np.zeros((T, E), np.float32)
    np.put_along_axis(gate_te, topk_idx, gates.astype(np.float32), axis=1)
    sel = np.zeros((T, E), bool)
    np.put_along_axis(sel, topk_idx, True, axis=1)
    return gate_te, sel, aux


def kernel(**inputs):
    x = np.asarray(inputs["x"], np.float32)
    Wr = np.asarray(inputs["Wr"], np.float32)
    W1 = np.asarray(inputs["W1"], np.float32)
    b1 = np.asarray(inputs["b1"], np.float32)
    W2 = np.asarray(inputs["W2"], np.float32)
    b2 = np.asarray(inputs["b2"], np.float32)
    top_k = int(np.asarray(inputs["top_k"]))

    B, S, D = x.shape
    E, F = W1.shape[0], W1.shape[1]
    T = B * S
    xf = x.reshape(T, D)
    assert E <= N_CORES, "one expert per core"

    gate_te, sel, aux = _route(xf, Wr, top_k)

    idx_lists = [np.nonzero(sel[:, e])[0] for e in range(E)]
    max_count = max(1, max(len(ix) for ix in idx_lists))
    # Per-round token capacity. MAX_C bounds SBUF residency (the xT tile is
    # C*2 bytes/partition); heavier imbalance just takes extra rounds.
    MAX_C = 1792
    rounds = -(-max_count // MAX_C)
    per_round = -(-max_count // rounds)
    C = min(-(-per_round // P) * P, MAX_C)

    w1t_maps = [np.ascontiguousarray(W1[e].T).astype(BF16_NP) for e in range(E)]
    w2t_maps = [np.ascontiguousarray(W2[e].T).astype(BF16_NP) for e in range(E)]
    zeros = {
        "xT": np.zeros((D, C), BF16_NP),
        "w1t": np.zeros((D, F), BF16_NP),
        "w2t": np.zeros((F, D), BF16_NP),
        "b1": np.zeros((F // P, P), np.float32),
        "gate": np.zeros((C // P, P), np.float32),
    }

    out_flat = np.zeros((T, D), np.float32)
    for r in range(rounds):
        ceff = max(
            [min(len(idx_lists[e]) - r * C, C) for e in range(E)] + [1]
        )
        nc = _get_program(D, F, C, ceff)
        in_maps = []
        for e in range(N_CORES):
            ix = idx_lists[e][r * C : (r + 1) * C] if e < E else []
            if len(ix) == 0:
                in_maps.append(zeros)
                continue
            n = len(ix)
            xTe = np.zeros((D, C), BF16_NP)
            xTe[:, :n] = xf[ix].T.astype(BF16_NP)
            g = np.zeros((C,), np.float32)
            g[:n] = gate_te[ix, e]
            in_maps.append(
                {
                    "xT": xTe,
                    "w1t": w1t_maps[e],
                    "w2t": w2t_maps[e],
                    "b1": np.ascontiguousarray(b1[e].reshape(F // P, P)),
                    "gate": g.reshape(C // P, P),
                }
            )
        res = _run_with_retry(nc, in_maps)
        for e in range(E):
            ix = idx_lists[e][r * C : (r + 1) * C]
            if len(ix) == 0:
                continue
            out_flat[ix] += res.results[e]["y"][: len(ix)]

    for e in range(E):
        ix = idx_lists[e]
        if len(ix) and np.any(b2[e]):
            out_flat[ix] += np.outer(gate_te[ix, e], b2[e])

    return out_flat.reshape(B, S, D), aux


# revision 39
# speedup vs baseline: 312.2657x; 1.0250x over previous
"""MoE layer (softmax router, top-k, per-expert FFN D->F->gelu->D) on 8 TRN2 cores.

Strategy: expert-parallel. The router (a tiny T x E matmul + top-k) runs on the
host and doubles as the sharding function: core e receives exactly the tokens
whose top-k set contains expert e (the "all-to-all dispatch"), already
transposed to (D, C) so both device matmuls need no on-chip transposes.
Each core holds one expert's weights (pre-transposed on host, cast to bf16),
computes gelu(x @ W1^T + b1) @ W2^T, scales each token's output by its
renormalized gate on-chip, and the host scatter-adds the per-expert results
back into the full (T, D) output. b2 and the aux load-balance loss are added
on the host (b2 enters linearly: sum_e gate * b2[e]).
"""

import numpy as np
import ml_dtypes

import concourse.bass as bass  # noqa: F401  (registers AP machinery)
import concourse.mybir as mybir
import concourse.tile as tile
from concourse import bacc
from concourse.bass_utils import run_bass_kernel_spmd

P = 128
CB = 512  # token-block: free dim of layer-1 matmuls, partition chunk of layer-2
LB_WEIGHT = 0.01
N_CORES = 8

BF16 = mybir.dt.bfloat16
F32 = mybir.dt.float32
BF16_NP = ml_dtypes.bfloat16

_PROGRAM_CACHE: dict = {}


def _token_blocks(C: int) -> list[int]:
    """Split C into ceil(C/CB) near-equal blocks, each a multiple of P.

    Balanced blocks keep every layer-1 matmul's free dim large (>=256 for any
    C >= 512) instead of leaving a 128-wide remainder block whose matmuls pay
    proportionally more issue/LDWEIGHTS overhead on hardware.
    """
    nb = -(-C // CB)
    base = C // nb // P * P
    blocks = [base] * nb
    extra = (C - base * nb) // P
    for i in range(extra):
        blocks[i] += P
    return blocks


def _build_program(
    D: int,
    F: int,
    C: int,
    ceff: int | None = None,  # true token count; layer 1 computes only these columns
    repeat: int = 1,
    psa_bufs: int = 3,
    psb_bufs: int = 2,
    y_bufs: int = 3,
    w2_late: bool = False,
    ht_split: int = 1,
    w1_chunk: int = 5,  # if >0, split each w1 k-tile into column chunks of this many m-tiles
    small_first: bool = False,  # schedule the remainder block first (startup is DMA-bound)
):
    """One-expert FFN over C (padded) tokens; SPMD-identical across cores.

    ceff < C skips layer-1 compute for the pad columns: layer 2 then reads
    uninitialized hT columns for tokens >= ceff, producing garbage only in
    output rows >= ceff, which carry gate 0 and are discarded by the host.
    """
    if ceff is None:
        ceff = C
    assert D % P == 0 and F % P == 0 and C % P == 0 and D % 512 == 0
    assert 0 < ceff <= C
    KD, MF, ND = D // P, F // P, D // 512
    offs, c0 = [], 0
    for cb in _token_blocks(C):
        offs.append((c0, cb))
        c0 += cb
    if small_first:
        offs.sort(key=lambda t: t[1])
    blocks = offs * repeat
    GELU = mybir.ActivationFunctionType.Gelu_apprx_tanh

    nc = bacc.Bacc(None, target_bir_lowering=False, debug=False)
    xT = nc.declare_dram_parameter("xT", [D, C], BF16, isOutput=False)
    w1t = nc.declare_dram_parameter("w1t", [D, F], BF16, isOutput=False)
    w2t = nc.declare_dram_parameter("w2t", [F, D], BF16, isOutput=False)
    b1 = nc.declare_dram_parameter("b1", [MF, P], F32, isOutput=False)
    # Output is yT (D, C): tokens on the matmul free axis, so layer 2 computes
    # only ceff real columns (no 128-padding waste); gate scale happens on host.
    yT = nc.declare_dram_parameter("yT", [D, C], F32, isOutput=True)

    w1v = w1t.rearrange("(k p) f -> k p f", p=P)
    w2v = w2t.rearrange("(k p) d -> k p d", p=P)
    xv = xT.rearrange("(k p) c -> k p c", p=P)
    yv = yT.rearrange("(k p) c -> k p c", p=P)

    with tile.TileContext(nc) as tc:
        with (
            tc.tile_pool(name="const", bufs=1) as constp,
            tc.tile_pool(name="wres", bufs=1) as wres,
            tc.tile_pool(name="xres", bufs=1) as xres,
            tc.tile_pool(name="htp", bufs=1) as htp,
            tc.tile_pool(name="ysb", bufs=y_bufs) as yp,
            tc.tile_pool(name="psA", bufs=psa_bufs, space="PSUM") as psA,
            tc.tile_pool(name="psB", bufs=psb_bufs, space="PSUM") as psB,
        ):
            b1_sb = constp.tile([P, MF], F32, tag="b1")
            nc.sync.dma_start(out=b1_sb[:], in_=b1.rearrange("m p -> p m"))

            x_sb = []
            for k in range(KD):
                t = xres.tile([P, C], BF16, tag=f"x{k}")
                nc.sync.dma_start(out=t[:], in_=xv[k])
                x_sb.append(t)
            if w1_chunk:
                nch = -(-MF // w1_chunk)
                w1_sb = [[None] * nch for _ in range(KD)]
                for ch in range(nch):
                    for k in range(KD):
                        mw = min(w1_chunk, MF - ch * w1_chunk) * P
                        t = wres.tile([P, mw], BF16, tag=f"w1_{k}_{ch}", name=f"w1_{k}_{ch}")
                        nc.sync.dma_start(
                            out=t[:], in_=w1v[k][:, ch * w1_chunk * P :][:, :mw]
                        )
                        w1_sb[k][ch] = t

                def w1_slice(k, m):
                    return w1_sb[k][m // w1_chunk][
                        :, (m % w1_chunk) * P : (m % w1_chunk + 1) * P
                    ]
            else:
                w1_sb = []
                for k in range(KD):
                    t = wres.tile([P, F], BF16, tag=f"w1_{k}")
                    nc.sync.dma_start(out=t[:], in_=w1v[k])
                    w1_sb.append(t)

                def w1_slice(k, m):
                    return w1_sb[k][:, m * P : (m + 1) * P]
            w2_sb = []

            def load_w2():
                for k in range(MF):
                    t = wres.tile([P, D], BF16, tag=f"w2_{k}")
                    nc.sync.dma_start(out=t[:], in_=w2v[k])
                    w2_sb.append(t)

            if not w2_late:
                load_w2()

            MS = MF // ht_split  # m-tiles per ht sub-tile
            CBMAX = max(cb for _, cb in blocks)
            for bi, (c0, cb) in enumerate(blocks):
                # Layer 1: hT[f, c] = gelu(sum_d w1t[d, f] * xT[d, c] + b1[f])
                # Only the first ce (true-token) columns of the block are computed.
                ce = min(cb, max(ceff - c0, 1))
                hts = [
                    htp.tile([P, MS, CBMAX], BF16, tag=f"ht{s}", name=f"ht{s}")
                    for s in range(ht_split)
                ]
                for m in range(MF):
                    ps = psA.tile([P, CBMAX], F32, tag="psA")
                    for k in range(KD):
                        nc.tensor.matmul(
                            ps[:, :ce],
                            w1_slice(k, m),
                            x_sb[k][:, c0 : c0 + ce],
                            start=(k == 0),
                            stop=(k == KD - 1),
                        )
                    nc.scalar.activation(
                        hts[m // MS][:, m % MS, :ce],
                        ps[:, :ce],
                        GELU,
                        bias=b1_sb[:, m : m + 1],
                    )
                if bi == 0 and w2_late:
                    load_w2()
                # Layer 2: yT[d, c] = sum_f w2t[f, d] * ht[f, c]  (ce columns only)
                for m2 in range(KD):
                    ps2 = psB.tile([P, CBMAX], F32, tag="psB")
                    for k in range(MF):
                        nc.tensor.matmul(
                            ps2[:, :ce],
                            w2_sb[k][:, m2 * P : (m2 + 1) * P],
                            hts[k // MS][:, k % MS, :ce],
                            start=(k == 0),
                            stop=(k == MF - 1),
                        )
                    ysb = yp.tile([P, CBMAX], F32, tag="y")
                    nc.vector.tensor_copy(ysb[:, :ce], ps2[:, :ce])
                    nc.sync.dma_start(
                        out=yv[m2][:, c0 : c0 + ce], in_=ysb[:, :ce]
                    )
    nc.compile()
    return nc


def _get_program(D: int, F: int, C: int, ceff: int):
    key = (D, F, C, ceff)
    if key not in _PROGRAM_CACHE:
        _PROGRAM_CACHE[key] = _build_program(D, F, C, ceff=ceff)
    return _PROGRAM_CACHE[key]


def _run_with_retry(nc, in_maps, attempts: int = 3):
    """Run the SPMD kernel; on a poisoned device session (e.g. transient
    NRT_EXEC_UNIT_UNRECOVERABLE through the axon tunnel) reset the PJRT
    backend — the in-process equivalent of a fresh session — and retry."""
    import time as _time

    for attempt in range(attempts):
        try:
            return run_bass_kernel_spmd(nc, in_maps, core_ids=list(range(N_CORES)))
        except Exception:
            if attempt == attempts - 1:
                raise
            try:
                import jax

                jax.clear_backends()
            except Exception:
                pass
            _time.sleep(3.0)


def _route(xf, Wr, top_k):
    """Host router mirroring the reference: softmax -> top-k -> renormalize."""
    T, E = xf.shape[0], Wr.shape[0]
    logits = (xf @ Wr.T).astype(np.float64)  # fp32 BLAS, then fp64 softmax
    logits -= logits.max(axis=1, keepdims=True)
    p = np.exp(logits)
    p /= p.sum(axis=1, keepdims=True)
    order = np.argsort(-p, axis=1, kind="stable")  # ties -> lower index, like jax
    topk_idx = order[:, :top_k]
    topk_p = np.take_along_axis(p, topk_idx, axis=1)
    gates = topk_p / topk_p.sum(axis=1, keepdims=True)

    usage = p.mean(axis=0)
    aux = np.float32(LB_WEIGHT * np.sum((usage - 1.0 / E) ** 2))

    gate_te = np.zeros((T, E), np.float32)
    np.put_along_axis(gate_te, topk_idx, gates.astype(np.float32), axis=1)
    sel = np.zeros((T, E), bool)
    np.put_along_axis(sel, topk_idx, True, axis=1)
    return gate_te, sel, aux


def kernel(**inputs):
    x = np.asarray(inputs["x"], np.float32)
    Wr = np.asarray(inputs["Wr"], np.float32)
    W1 = np.asarray(inputs["W1"], np.float32)
    b1 = np.asarray(inputs["b1"], np.float32)
    W2 = np.asarray(inputs["W2"], np.float32)
    b2 = np.asarray(inputs["b2"], np.float32)
    top_k = int(np.asarray(inputs["top_k"]))

    B, S, D = x.shape
    E, F = W1.shape[0], W1.shape[1]
    T = B * S
    xf = x.reshape(T, D)
    assert E <= N_CORES, "one expert per core"

    gate_te, sel, aux = _route(xf, Wr, top_k)

    idx_lists = [np.nonzero(sel[:, e])[0] for e in range(E)]
    max_count = max(1, max(len(ix) for ix in idx_lists))
    # Per-round token capacity. MAX_C bounds SBUF residency (the xT tile is
    # C*2 bytes/partition); heavier imbalance just takes extra rounds.
    MAX_C = 1792
    rounds = -(-max_count // MAX_C)
    per_round = -(-max_count // rounds)
    C = min(-(-per_round // P) * P, MAX_C)

    w1t_maps = [np.ascontiguousarray(W1[e].T).astype(BF16_NP) for e in range(E)]
    w2t_maps = [np.ascontiguousarray(W2[e].T).astype(BF16_NP) for e in range(E)]
    zeros = {
        "xT": np.zeros((D, C), BF16_NP),
        "w1t": np.zeros((D, F), BF16_NP),
        "w2t": np.zeros((F, D), BF16_NP),
        "b1": np.zeros((F // P, P), np.float32),
    }

    out_flat = np.zeros((T, D), np.float32)
    for r in range(rounds):
        ceff = max(
            [min(len(idx_lists[e]) - r * C, C) for e in range(E)] + [1]
        )
        nc = _get_program(D, F, C, ceff)
        in_maps = []
        for e in range(N_CORES):
            ix = idx_lists[e][r * C : (r + 1) * C] if e < E else []
            if len(ix) == 0:
                in_maps.append(zeros)
                continue
            n = len(ix)
            xTe = np.zeros((D, C), BF16_NP)
            xTe[:, :n] = xf[ix].T.astype(BF16_NP)
            in_maps.append(
                {
                    "xT": xTe,
                    "w1t": w1t_maps[e],
                    "w2t": w2t_maps[e],
                    "b1": np.ascontiguousarray(b1[e].reshape(F // P, P)),
                }
            )
        res = _run_with_retry(nc, in_maps)
        for e in range(E):
            ix = idx_lists[e][r * C : (r + 1) * C]
            if len(ix) == 0:
                continue
            # yT is (D, C); apply the gate here (same fp32 multiply the
            # reference's combine step performs, same ascending-e add order).
            out_flat[ix] += gate_te[ix, e][:, None] * res.results[e]["yT"][:, : len(ix)].T

    for e in range(E):
        ix = idx_lists[e]
        if len(ix) and np.any(b2[e]):
            out_flat[ix] += np.outer(gate_te[ix, e], b2[e])

    return out_flat.reshape(B, S, D), aux
